# revision 14
# baseline (speedup 1.0000x reference)
"""Raw-bacc version of the L1-distance classifier kernel (no TileContext).

Same algorithm as kernel.py (sign-trick + binned |w| correction via fp8
DoubleRow matmuls) but with hand-placed engines and semaphores to avoid
the Tile framework's preamble/tail overhead.

Engine plan:
  sync   : DMA issue (x halves A, corr groups 0/2, main 0, out 0/2) + end wait
  scalar : DMA issue (x halves B, corr 1/3, main 1, out 1/3), bf16 casts,
           evictions for b-tiles 0/2 (Identity + negA bias)
  tensor : 8 transposes, then 16 bf16 main matmuls + 64 fp8 DoubleRow matmuls
  vector : transpose-copies, feature planes, |x| row-sum, evictions 1/3
  gpsimd : identity matrix only
"""

import os

import ml_dtypes
import numpy as np

import concourse.bass as bass
import concourse.mybir as mybir
from concourse import bacc
from concourse.bass_utils import run_bass_kernel_spmd

BATCH, N_CLASSES, INPUT_DIM = 4096, 512, 256
N_CORES = 8
BL = BATCH // N_CORES
P = 128
B_TILES = BL // P                # 4
D_TILES = INPUT_DIM // P         # 2
M_BINS = 6
N_CORR = 2 * M_BINS
CORR_G = 4
N_CG = N_CORR // CORR_G

F32 = mybir.dt.float32
BF16 = mybir.dt.bfloat16
FP8 = mybir.dt.float8e4
OP = mybir.AluOpType
AF = mybir.ActivationFunctionType

LAST_RUN = None
_CACHE = {}
_IDENT = np.eye(128, dtype=ml_dtypes.bfloat16)


def _build_graph(vc):
    nc = bacc.Bacc(None, target_bir_lowering=False)
    x_dram = nc.declare_dram_parameter("x", [B_TILES, P, INPUT_DIM], F32, isOutput=False)
    rhsm_dram = nc.declare_dram_parameter(
        "rhs_main", [2, D_TILES, P, N_CLASSES], BF16, isOutput=False
    )
    rhsc_dram = nc.declare_dram_parameter(
        "rhs_corr", [N_CG, CORR_G, P, D_TILES * N_CLASSES], FP8, isOutput=False
    )
    ident_dram = nc.declare_dram_parameter("ident", [P, P], BF16, isOutput=False)
    out_dram = nc.declare_dram_parameter("out", [BL, N_CLASSES], F32, isOutput=True)

    from contextlib import ExitStack
    with ExitStack() as _ctx:
        ident = _ctx.enter_context(nc.sbuf_tensor("ident_sb", [P, P], BF16))
        x_all = _ctx.enter_context(nc.sbuf_tensor("x_all", [P, B_TILES, INPUT_DIM], F32))
        xb_all = _ctx.enter_context(nc.sbuf_tensor("xb_all", [P, B_TILES, INPUT_DIM], BF16))
        xTb = _ctx.enter_context(nc.sbuf_tensor("xTb", [P, D_TILES, BL], BF16))
        pos = _ctx.enter_context(nc.sbuf_tensor("pos", [P, D_TILES, BL], BF16))
        negp = _ctx.enter_context(nc.sbuf_tensor("negp", [P, D_TILES, BL], BF16))
        corrpl = _ctx.enter_context(nc.sbuf_tensor("corrpl", [P, N_CORR, D_TILES, BL], FP8))
        rm = _ctx.enter_context(nc.sbuf_tensor("rm", [P, 2, D_TILES, N_CLASSES], BF16))
        rc = _ctx.enter_context(nc.sbuf_tensor("rc", [P, N_CG, CORR_G, D_TILES * N_CLASSES], FP8))
        na = _ctx.enter_context(nc.sbuf_tensor("na", [P, B_TILES], F32))
        osb = _ctx.enter_context(nc.sbuf_tensor("osb", [P, B_TILES, N_CLASSES], F32))
        acc = [
            _ctx.enter_context(nc.psum_tensor(f"acc{i}", [P, N_CLASSES], F32))
            for i in range(B_TILES)
        ]
        tp = [
            _ctx.enter_context(nc.psum_tensor(f"tp{i}", [P, P], BF16)) for i in range(2)
        ]
        s_x = [_ctx.enter_context(nc.semaphore(f"s_x{i}")) for i in range(B_TILES)]
        s_rm = [_ctx.enter_context(nc.semaphore(f"s_rm{i}")) for i in range(2)]
        s_rc = [_ctx.enter_context(nc.semaphore(f"s_rc{i}")) for i in range(N_CG)]
        s_id = _ctx.enter_context(nc.semaphore("s_id"))
        s_cast = _ctx.enter_context(nc.semaphore("s_cast"))
        s_tp = _ctx.enter_context(nc.semaphore("s_tp"))
        s_tpc = _ctx.enter_context(nc.semaphore("s_tpc"))
        s_feat = _ctx.enter_context(nc.semaphore("s_feat"))
        s_feat2 = _ctx.enter_context(nc.semaphore("s_feat2"))
        s_mm = _ctx.enter_context(nc.semaphore("s_mm"))
        s_na = _ctx.enter_context(nc.semaphore("s_na"))
        s_ev_e = _ctx.enter_context(nc.semaphore("s_ev_e"))
        s_ev_o = _ctx.enter_context(nc.semaphore("s_ev_o"))
        s_out = _ctx.enter_context(nc.semaphore("s_out"))
        s_out2 = _ctx.enter_context(nc.semaphore("s_out2"))

        HP = P // 2  # partition half

        with nc.Block() as block:

            @block.sync
            def _(sync):
                sync.dma_start(out=ident[:], in_=ident_dram[:]).then_inc(s_id, 16)
                for bt in (0, 2):
                    sync.dma_start(out=x_all[:, bt, :], in_=x_dram[bt]).then_inc(
                        s_x[bt], 16
                    )
                for bt in range(B_TILES):
                    sync.wait_ge(s_x[bt], 16)
                sync.dma_start(
                    out=rm[:, 0, :, :], in_=rhsm_dram[0].rearrange("t p c -> p t c")
                ).then_inc(s_rm[0], 16)
                for g in (0, 1):
                    sync.dma_start(
                        out=rc[:, g, :, :],
                        in_=rhsc_dram[g].rearrange("j p c -> p j c"),
                    ).then_inc(s_rc[g], 16)
                pairs = [(s_ev_e, 1, 0), (s_ev_o, 1, 1), (s_ev_e, 2, 2), (s_ev_o, 2, 3)]
                for sem, val, bt in pairs:
                    sync.wait_ge(sem, val)
                    sync.dma_start(
                        out=out_dram[bt * P : (bt + 1) * P, :], in_=osb[:, bt, :]
                    ).then_inc(s_out, 16)
                sync.wait_ge(s_out, 64)

            @block.scalar
            def _(scalar):
                for bt in (1, 3):
                    scalar.dma_start(out=x_all[:, bt, :], in_=x_dram[bt]).then_inc(
                        s_x[bt], 16
                    )

                scalar.dma_start(
                    out=rm[:, 1, :, :], in_=rhsm_dram[1].rearrange("t p c -> p t c")
                ).then_inc(s_rm[1], 16)
                for g in range(2, N_CG):
                    scalar.dma_start(
                        out=rc[:, g, :, :],
                        in_=rhsc_dram[g].rearrange("j p c -> p j c"),
                    ).then_inc(s_rc[g], 16)
                scalar.wait_ge(s_na, 1)
                for i, bt in enumerate((0, 2)):
                    scalar.wait_ge(s_mm, bt + 1)
                    scalar.activation(
                        out=osb[:, bt, :], in_=acc[bt][:], func=AF.Identity,
                        bias=na[:, bt : bt + 1], scale=1.0,
                    ).then_inc(s_ev_e, 1)

            @block.tensor
            def _(tensor):
                tensor.wait_ge(s_id, 16)
                # transposes t-major so the t=0 planes complete first
                i = 0
                for t in range(D_TILES):
                    for bt in range(B_TILES):
                        tensor.wait_ge(s_cast, bt + 1)
                        if i >= 2:
                            tensor.wait_ge(s_tpc, i - 1)
                        tensor.transpose(
                            tp[i % 2][:], xb_all[:, bt, t * P : (t + 1) * P], ident[:]
                        ).then_inc(s_tp, 1)
                        i += 1
                mains = [pos, negp]
                tensor.wait_ge(s_rm[0], 16)
                tensor.wait_ge(s_rm[1], 16)
                for t in range(D_TILES):
                    for p in range(2):
                        tensor.wait_ge(s_feat, 2 * t + p + 1)
                        for bt in range(B_TILES):
                            tensor.matmul(
                                acc[bt][:],
                                mains[p][:, t, bt * P : (bt + 1) * P],
                                rm[:, p, t, :],
                                start=(p == 0 and t == 0),
                                stop=False,
                            )
                for j in range(N_CORR):
                    tensor.wait_ge(s_feat2, j + 1)
                    g, jj = divmod(j, CORR_G)
                    if jj == 0:
                        tensor.wait_ge(s_rc[g], 16)
                    rcj = rc[:, g, jj, :].rearrange("p (t c) -> p t c", t=D_TILES)
                    for bt in range(B_TILES):
                        mm = tensor.matmul(
                            acc[bt][:],
                            corrpl[:, j, :, bt * P : (bt + 1) * P],
                            rcj,
                            start=False,
                            stop=(j == N_CORR - 1),
                            perf_mode=mybir.MatmulPerfMode.DoubleRow,
                        )
                        if j == N_CORR - 1:
                            mm.then_inc(s_mm, 1)

            @block.vector
            def _(vector):
                for bt in range(B_TILES):
                    vector.wait_ge(s_x[bt], 16)
                    vector.tensor_copy(
                        xb_all[:, bt, :], x_all[:, bt, :]
                    ).then_inc(s_cast, 1)
                # copies in t-major transpose order; t0 planes first, then
                # cp0 before the t1 planes so the first DR matmul isn't gated
                i = 0
                for t in range(D_TILES):
                    for bt in range(B_TILES):
                        vector.wait_ge(s_tp, i + 1)
                        vector.tensor_copy(
                            xTb[:, t, bt * P : (bt + 1) * P], tp[i % 2][:]
                        ).then_inc(s_tpc, 1)
                        i += 1
                    if t == 0:
                        vector.wait_ge(s_tpc, B_TILES)
                        vector.tensor_scalar(
                            out=pos[:, 0, :], in0=xTb[:, 0, :], scalar1=0.0,
                            scalar2=None, op0=OP.is_gt,
                        ).then_inc(s_feat, 1)
                        vector.tensor_scalar(
                            out=negp[:, 0, :], in0=xTb[:, 0, :], scalar1=0.0,
                            scalar2=None, op0=OP.is_lt,
                        ).then_inc(s_feat, 1)
                vector.wait_ge(s_tpc, 2 * B_TILES)
                vector.tensor_scalar(
                    out=corrpl[:, 0, :, :], in0=xTb[:, :, :],
                    scalar1=0.0, scalar2=float(vc[0]), op0=OP.max, op1=OP.min,
                ).then_inc(s_feat2, 1)
                vector.tensor_scalar(
                    out=pos[:, 1, :], in0=xTb[:, 1, :], scalar1=0.0, scalar2=None,
                    op0=OP.is_gt,
                ).then_inc(s_feat, 1)
                vector.tensor_scalar(
                    out=negp[:, 1, :], in0=xTb[:, 1, :], scalar1=0.0, scalar2=None,
                    op0=OP.is_lt,
                ).then_inc(s_feat, 1)
                for j in range(1, M_BINS):
                    vector.tensor_scalar(
                        out=corrpl[:, j, :, :], in0=xTb[:, :, :],
                        scalar1=0.0, scalar2=float(vc[j]), op0=OP.max, op1=OP.min,
                    ).then_inc(s_feat2, 1)
                for j in range(M_BINS):
                    vector.tensor_scalar(
                        out=corrpl[:, M_BINS + j, :, :], in0=xTb[:, :, :],
                        scalar1=0.0, scalar2=float(-vc[j]), op0=OP.min, op1=OP.max,
                    ).then_inc(s_feat2, 1)
                for bt in range(B_TILES):
                    vector.wait_ge(s_x[bt], 16)
                vector.tensor_reduce(
                    out=na[:], in_=x_all[:], axis=mybir.AxisListType.X,
                    op=OP.add, apply_absolute_value=True, negate=True,
                ).then_inc(s_na, 1)
                vector.wait_ge(s_na, 1)
                for bt in (1, 3):
                    vector.wait_ge(s_mm, bt + 1)
                    vector.tensor_scalar(
                        out=osb[:, bt, :], in0=acc[bt][:],
                        scalar1=na[:, bt : bt + 1], scalar2=None, op0=OP.add,
                    ).then_inc(s_ev_o, 1)

    nc.compile()
    return nc


def _host_prep(W, b):
    C, D = W.shape
    v = np.abs(W)
    vmax = float(v.max()) * 1.000001 + 1e-12
    delta = vmax / M_BINS
    vc = (np.arange(M_BINS) + 0.5) * delta
    bin_idx = np.minimum((v / delta).astype(np.int32), M_BINS - 1)
    vcw = vc[bin_idx].astype(np.float32)
    psi_p = np.where(W > 0, vcw, 0.0).astype(np.float32)
    psi_n = np.where(W < 0, vcw, 0.0).astype(np.float32)
    bias = (b / D)[:, None].astype(np.float32)

    main = np.stack([(W - 2 * psi_p + bias).T, (-W - 2 * psi_n + bias).T])
    rhs_main = np.ascontiguousarray(main).reshape(2, D_TILES, P, C)
    rhs_main = rhs_main.astype(ml_dtypes.bfloat16)

    corr = np.empty((N_CORR, D, C), dtype=np.float32)
    for j in range(M_BINS):
        corr[j] = (2.0 * ((W > 0) & (bin_idx == j))).T
        corr[M_BINS + j] = (-2.0 * ((W < 0) & (bin_idx == j))).T
    corr = corr.reshape(N_CORR, D_TILES, P, C).transpose(0, 2, 1, 3)
    corr = corr.reshape(N_CG, CORR_G, P, D_TILES * C)
    rhs_corr = np.ascontiguousarray(corr).astype(ml_dtypes.float8_e4m3)
    return vc, rhs_main, rhs_corr


def kernel(x, W, b):
    global LAST_RUN
    x = np.ascontiguousarray(np.asarray(x, dtype=np.float32))
    W = np.ascontiguousarray(np.asarray(W, dtype=np.float32))
    b = np.ascontiguousarray(np.asarray(b, dtype=np.float32))
    assert x.shape == (BATCH, INPUT_DIM) and W.shape == (N_CLASSES, INPUT_DIM)

    vc, rhs_main, rhs_corr = _host_prep(W, b)
    key = tuple(np.round(vc, 9).tolist())
    nc = _CACHE.get(key)
    if nc is None:
        nc = _build_graph(vc)
        _CACHE[key] = nc

    in_maps = [
        {
            "x": np.ascontiguousarray(
                x[i * BL : (i + 1) * BL].reshape(B_TILES, P, INPUT_DIM)
            ),
            "rhs_main": rhs_main,
            "rhs_corr": rhs_corr,
            "ident": _IDENT,
        }
        for i in range(N_CORES)
    ]
    LAST_RUN = run_bass_kernel_spmd(
        nc,
        in_maps,
        list(range(N_CORES)),
        trace=bool(int(os.environ.get("KERNEL_TRACE", "0"))),
    )
    out = np.concatenate(
        [np.asarray(LAST_RUN.results[i]["out"]) for i in range(N_CORES)], axis=0
    )
    return out.astype(np.float32)
